# revision 8
# baseline (speedup 1.0000x reference)
"""CTC prefix-scorer kernel for 8 Trainium2 NeuronCores (Bass/Tile).

Math: the reference's scan collapses because gamma_n_g is statically NEG_INF,
so phi_t == B[t-1] (cumsum of blank log-probs) and the n/b carries are dead.
The output is
    score[j] = logsumexp_{t=9..T-1}( B[t-1] - lse[t] + ctc_prob[t, c[j]] )
    score[j] = B[T-1]                       if c[j] == EOS (==1)
    out      = score.reshape(N, ctc_beam)
which depends on j only through c[j], so we compute a per-vocab table S[v]
on-device and gather on host.

Single fused launch, T-sharded: core k owns 188 contiguous time rows over the
FULL vocab, stored transposed (vocab on partitions) in v-block-major order:
    xg[p, (g*250 + b)*47 + j] = x[t0_k + 47 g + j, 128 b + p]   (fp8 e4m3)
Per t-group g (47 rows) the device computes, with no cross-core traffic:
    E = exp(x)                          ACT, fp8 in -> bf16, the 39 us floor
    rowsum[t] = sum_v E                 PE ones-matmuls (25 slabs of 470 cols
                                        accumulated in PSUM) + DVE 2nd stage
    lse = Ln(rowsum)                    ACT
    u = exclcumsum(xb - lse) - lse + mask - max(...)   tiny [1,47] DVE chain
    coeff = exp(u)                      ACT; replicated to 128 partitions via
                                        a rank-1 ones matmul
    cs[p, b] = sum_j coeff[j] E[p,b,j]  DVE in-place multiply + 3D X-reduce
Groups are mathematically independent: each group's coefficients are
normalized by its own running max m_g, and the host rescales each (core,
group) partial by alpha = exp(B[tstart-1] + m_g - K) when summing, so the
cross-group/core cumsum coupling lives entirely on the host (f64).
"""

import os
import numpy as np

# ---- problem constants (hardcoded per contract) ----
T = 1500
V = 32000
N_BATCH = 8
CTC_BEAM = 2048
N_CORES = 8
EOS = 1
BLANK_COL = V - 1
NEG_BIG = np.float32(-1e30)

TL = 188                 # time rows per core
NG = 4                   # t-groups per core
GT = TL // NG            # 47 rows per group
VB = 250                 # vocab blocks of 128
VBW = 128
GFREE = VB * GT          # 11750 free elements per group
SLAB = 10                # vocab blocks per rowsum matmul
NSLAB = VB // SLAB       # 25
SLABW = SLAB * GT        # 470 moving columns per matmul
GLEN = 9                 # U-1, static in the reference

# core k owns rows [T0[k], T0[k]+188); core 7 overlaps core 6 by 4 rows
T0 = [188 * k for k in range(7)] + [T - TL]  # [..., 1312]

_CACHE: dict = {}
LAST_EXEC_TIMES: list = []


def _limit_dma_sem_lanes():
    """Walrus on this toolchain accepts very few semaphore waits per
    instruction; track each DMA family on a single counting semaphore."""
    import concourse.tile_sem_assignment as tsa

    tsa.NUM_SWDGE_GLOBAL_SEMS = 1
    tsa.NUM_HWDGE_SEMS = 1


def _patch_drain_split():
    """Split the kernel-tail drain's wait list across a chain of single-wait
    drains (walrus accepts one semaphore wait per instruction)."""
    import bass_rust
    import concourse.tile as tile_mod
    from concourse.vector_clock import ScopedClock

    if getattr(tile_mod.TileContext, "_drain_split_patched", False):
        return

    def _drain_and_barrier(self, tick_clock, wait_clock):
        drain_inst = self.nc.scalar.drain()
        wait_clock.add_sem_waits(
            drain_inst.ins, ScopedClock({None: tick_clock.global_clock})
        )
        si = drain_inst.ins.sync_info
        waits = list(si.on_wait) if si is not None else []
        if len(waits) > 1:
            drain_inst.ins.sync_info = bass_rust.SyncInfo(
                on_wait=[waits[0]], on_update=list(si.on_update)
            )
            for wt in waits[1:]:
                extra = self.nc.sync.drain()
                extra.ins.sync_info = bass_rust.SyncInfo(
                    on_wait=[wt], on_update=[]
                )

        self.nc.all_engine_barrier()
        assert self.sems is not None
        popped = self.nc._tile_sem_poison_stack.pop()
        assert popped is self._sem_poison
        self.nc.clear_and_free_semaphores(list(self.sems.allocated().values()))
        self.nc.all_engine_barrier()

    tile_mod.TileContext._drain_and_barrier = _drain_and_barrier
    tile_mod.TileContext._drain_split_patched = True


def _build_nc():
    import concourse.bass as bass
    import concourse.tile as tile
    from concourse import mybir

    _limit_dma_sem_lanes()
    _patch_drain_split()
    nc = bass.Bass()
    xg = nc.dram_tensor("xg", [VBW, NG * GFREE], mybir.dt.float8e4,
                        kind="ExternalInput")
    xb = nc.dram_tensor("xb", [1, TL], mybir.dt.float32, kind="ExternalInput")
    mk = nc.dram_tensor("mk", [1, TL], mybir.dt.float32, kind="ExternalInput")
    rs = nc.dram_tensor("rs", [1, TL + NG], mybir.dt.float32,
                        kind="ExternalOutput")
    cs = nc.dram_tensor("cs", [VBW, NG * VB], mybir.dt.bfloat16,
                        kind="ExternalOutput")

    CUMLEVELS = [1, 2, 4, 8, 16, 32]   # Hillis-Steele spans 63 >= GT
    PAD = 64                           # zero prefix for shifted reads

    with tile.TileContext(nc) as tc:
        with (
            tc.tile_pool(name="sing", bufs=1) as sing,
            tc.tile_pool(name="psp", space="PSUM", bufs=1) as psp,
        ):
            ones_pe = nc.const_aps.tensor(1.0, (VBW, 1), mybir.dt.bfloat16)
            ones_rep = nc.const_aps.tensor(1.0, (1, VBW), mybir.dt.float32)

            xb_sb = sing.tile([1, TL], mybir.dt.float32)
            mk_sb = sing.tile([1, TL], mybir.dt.float32)
            xb2 = sing.tile([1, TL], mybir.dt.float32)
            mk2 = sing.tile([1, TL], mybir.dt.float32)
            rsb = sing.tile([1, TL + NG], mybir.dt.float32)
            lse_sb = sing.tile([1, TL], mybir.dt.float32)
            bl_sb = sing.tile([1, TL], mybir.dt.float32)
            cA = sing.tile([1, PAD + GT], mybir.dt.float32)
            cB = sing.tile([1, PAD + GT], mybir.dt.float32)
            w1 = sing.tile([1, TL], mybir.dt.float32)
            m_sb = sing.tile([1, NG], mybir.dt.float32)
            co_sb = sing.tile([1, TL], mybir.dt.float32)
            crep = sing.tile([VBW, TL], mybir.dt.bfloat16)
            csb = sing.tile([VBW, NG * VB], mybir.dt.bfloat16)
            probe = sing.tile([1, 1], mybir.dt.bfloat16)

            xt = [
                sing.tile([VBW, GFREE], mybir.dt.float8e4,
                          name=f"xt{g}", tag=f"xt{g}")
                for g in range(NG)
            ]
            et = [
                sing.tile([VBW, GFREE], mybir.dt.bfloat16,
                          name=f"et{g}", tag=f"et{g}")
                for g in range(NG)
            ]
            ps_rs = [
                psp.tile([1, SLABW], mybir.dt.float32,
                         name=f"psr{g}", tag=f"psr{g}")
                for g in range(NG)
            ]
            ps_rep = [
                psp.tile([VBW, GT], mybir.dt.float32,
                         name=f"psp{g}", tag=f"psp{g}")
                for g in range(NG)
            ]

            # tiny inputs + x-slabs; group g's data rides queue g%2 so each
            # ACT exp waits on a single DMA semaphore lane
            nc.sync.dma_start(out=xb_sb, in_=xb[:, :])
            nc.sync.dma_start(out=mk_sb, in_=mk[:, :])
            for g in range(NG):
                eng = nc.sync if g % 2 == 0 else nc.gpsimd
                eng.dma_start(out=xt[g], in_=xg[:, g * GFREE:(g + 1) * GFREE])

            nc.vector.memset(cA[0:1, 0:PAD + 1], 0.0)
            nc.vector.memset(cB[0:1, 0:PAD + 1], 0.0)

            # absorb the tiny-input DMA semaphore into ACT's observed clock:
            # every DVE read of xb/mk below is then covered by the single
            # ACT-clock wait each DVE chain already carries
            nc.scalar.copy(out=xb2, in_=xb_sb)
            nc.scalar.copy(out=mk2, in_=mk_sb)

            for g in range(NG):
                sl = slice(g * GT, (g + 1) * GT)
                nc.scalar.activation(
                    out=et[g], in_=xt[g],
                    func=mybir.ActivationFunctionType.Exp,
                )
                for s in range(NSLAB):
                    nc.tensor.matmul(
                        ps_rs[g][0:1, :],
                        ones_pe,
                        et[g][:, s * SLABW:(s + 1) * SLABW],
                        start=(s == 0),
                        stop=(s == NSLAB - 1),
                    )
                # [1, 470] = 10 slab-interleaved partial rowsums; fold the
                # slab dim (stride GT, innermost) on DVE
                nc.vector.tensor_reduce(
                    out=rsb[0:1, sl],
                    in_=ps_rs[g].rearrange("p (b j) -> p j b", b=SLAB, j=GT),
                    axis=mybir.AxisListType.X,
                    op=mybir.AluOpType.add,
                )
                nc.scalar.activation(
                    out=lse_sb[0:1, sl], in_=rsb[0:1, sl],
                    func=mybir.ActivationFunctionType.Ln,
                )
                nc.vector.tensor_tensor(
                    out=bl_sb[0:1, sl], in0=xb2[0:1, sl],
                    in1=lse_sb[0:1, sl], op=mybir.AluOpType.subtract,
                )
                # exclusive cumsum of bl within the group (Hillis-Steele over
                # a zero-padded prefix; shift-in by one implements exclusivity)
                nc.vector.tensor_copy(
                    out=cA[0:1, PAD + 1:PAD + GT],
                    in_=bl_sb[0:1, g * GT:g * GT + GT - 1],
                )
                src, dst = cA, cB
                for k in CUMLEVELS:
                    nc.vector.tensor_tensor(
                        out=dst[0:1, PAD:PAD + GT],
                        in0=src[0:1, PAD:PAD + GT],
                        in1=src[0:1, PAD - k:PAD + GT - k],
                        op=mybir.AluOpType.add,
                    )
                    src, dst = dst, src
                cum = src
                # w1 = cum - lse + mask; m = max(w1); u = w1 - m
                nc.vector.tensor_tensor(
                    out=w1[0:1, sl], in0=cum[0:1, PAD:PAD + GT],
                    in1=lse_sb[0:1, sl], op=mybir.AluOpType.subtract,
                )
                nc.vector.tensor_tensor(
                    out=w1[0:1, sl], in0=w1[0:1, sl], in1=mk2[0:1, sl],
                    op=mybir.AluOpType.add,
                )
                nc.vector.tensor_reduce(
                    out=m_sb[0:1, g:g + 1], in_=w1[0:1, sl],
                    axis=mybir.AxisListType.X, op=mybir.AluOpType.max,
                )
                nc.vector.tensor_scalar(
                    out=w1[0:1, sl], in0=w1[0:1, sl],
                    scalar1=m_sb[0:1, g:g + 1], scalar2=None,
                    op0=mybir.AluOpType.subtract,
                )
                nc.scalar.activation(
                    out=co_sb[0:1, sl], in_=w1[0:1, sl],
                    func=mybir.ActivationFunctionType.Exp,
                )
                # replicate coeff across partitions: ones[1,128]^T @ co[1,47]
                nc.tensor.matmul(
                    ps_rep[g][:, :], ones_rep, co_sb[0:1, sl],
                    start=True, stop=True,
                )
                nc.scalar.copy(out=crep[:, sl], in_=ps_rep[g][:, :])
                # weighted column partials: in-place scale then X-reduce
                e3 = et[g].rearrange("p (b j) -> p b j", b=VB, j=GT)
                c3 = crep[:, sl].unsqueeze(1).broadcast_to((VBW, VB, GT))
                nc.vector.tensor_tensor(
                    out=e3, in0=e3, in1=c3, op=mybir.AluOpType.mult,
                )
                # DVE reduces in f32 internally; only the stored partial is
                # bf16 (needed for the 4x 2-byte DVE mode), and the host
                # rescale absorbs the rounding
                with nc.allow_low_precision(reason="bf16 column partials"):
                    nc.vector.tensor_reduce(
                        out=csb[:, g * VB:(g + 1) * VB], in_=e3,
                        axis=mybir.AxisListType.X, op=mybir.AluOpType.add,
                    )

            nc.scalar.copy(out=rsb[0:1, TL:TL + NG], in_=m_sb[0:1, :])
            # absorb the final DVE tick (last csb reduce) into the ACT clock
            # so the output DMAs below carry only their ring-lane wait
            nc.scalar.copy(out=probe, in_=csb[0:1, NG * VB - 1:NG * VB])
            nc.scalar.dma_start(out=rs[:, :], in_=rsb)
            nc.scalar.dma_start(out=cs[:, :], in_=csb)
    return nc


def _get_program():
    if "nc" not in _CACHE:
        _CACHE["nc"] = _build_nc()
    return _CACHE["nc"]


def _run_spmd(nc, in_maps):
    from concourse.bass_utils import run_bass_kernel_spmd

    trace = bool(int(os.environ.get("CTC_TRACE", "0")))
    if trace:
        try:
            res = run_bass_kernel_spmd(
                nc, in_maps, core_ids=list(range(N_CORES)), trace=True
            )
            LAST_EXEC_TIMES.append(res.exec_time_ns)
            return res.results
        except ModuleNotFoundError:
            pass
    res = run_bass_kernel_spmd(
        nc, in_maps, core_ids=list(range(N_CORES)), trace=False
    )
    return res.results


def kernel(ctc_prob, g, c):
    import ml_dtypes

    x = np.ascontiguousarray(np.asarray(ctc_prob, dtype=np.float32))
    c_np = np.asarray(c).astype(np.int64).ravel()

    nc = _get_program()

    x8 = x.astype(ml_dtypes.float8_e4m3)
    in_maps = []
    for k in range(N_CORES):
        t0 = T0[k]
        xs = x8[t0:t0 + TL]                      # [188, 32000]
        xgk = np.ascontiguousarray(
            xs.reshape(NG, GT, VB, VBW).transpose(3, 0, 2, 1)
            .reshape(VBW, NG * GFREE)
        )
        xbk = np.ascontiguousarray(
            x[t0:t0 + TL, BLANK_COL].reshape(1, TL)
        )
        mkk = np.zeros((1, TL), dtype=np.float32)
        for i in range(TL):
            t_glob = t0 + i
            if t_glob < GLEN:
                mkk[0, i] = NEG_BIG          # scan starts at t = 9
            if k == 7 and i < (T0[6] + TL) - T0[7]:
                mkk[0, i] = NEG_BIG          # rows duplicated from core 6
        in_maps.append({"xg": xgk, "xb": xbk, "mk": mkk})

    res = _run_spmd(nc, in_maps)

    # ---- host combine (f64) ----
    rowsum = np.empty(T, dtype=np.float64)
    for k in range(N_CORES):
        rowsum[T0[k]:T0[k] + TL] = res[k]["rs"][0, :TL].astype(np.float64)
    lse = np.log(rowsum)
    blank_lp = x[:, BLANK_COL].astype(np.float64) - lse
    B = np.cumsum(blank_lp)
    w = B[GLEN - 1:T - 1] - lse[GLEN:]           # w[t], t = 9..T-1
    K = float(w.max())

    colsum = np.zeros(V, dtype=np.float64)
    with np.errstate(under="ignore"):
        for k in range(N_CORES):
            csk = res[k]["cs"].astype(np.float64)    # [128, 1000]
            for gi in range(NG):
                tstart = T0[k] + gi * GT
                bprev = B[tstart - 1] if tstart > 0 else 0.0
                m = float(res[k]["rs"][0, TL + gi])
                alpha = np.exp(bprev + m - K)
                if alpha == 0.0:
                    continue
                part = csk[:, gi * VB:(gi + 1) * VB]  # [128 p, 250 b]
                colsum += alpha * part.T.ravel()      # v = 128 b + p

    S = K + np.log(colsum)
    score = S[c_np]
    score = np.where(c_np == EOS, B[T - 1], score)
    return score.astype(np.float32).reshape(N_BATCH, CTC_BEAM)
